# revision 1
# baseline (speedup 1.0000x reference)
"""MoE (top-2 of 8 experts, SwiGLU FFN) Trainium2 kernel, expert-parallel on 8 cores.

Strategy
--------
Host: router (softmax + top-2 + renorm, ~0.03% of FLOPs) decides the shard:
tokens are dispatched to their experts (all-to-all on the host side, which
owns sharding/gather per the harness contract). Core e receives the tokens
routed to expert e (padded to the max per-expert count C) plus expert e's
w1/w3/w2, pre-packed into PE-friendly layouts.

Device (per core, identical SPMD program):
  h1T[f,c] = sum_h w1[h,f]*xT[h,c]   (fp32r matmuls, K=128 subtiles)
  h3T likewise; actT = silu(h1T)*h3T  (ACT + DVE, PSUM->SBUF)
  outT[h,c] = sum_f w2[f,h]*actT[f,c]
F is processed in 4 groups of 11 f-tiles so actT never exceeds SBUF;
the w2 partial products accumulate into an SBUF outT across groups.
All matmuls keep the moving (token) dim as 2 chunks ~C/2 >= 256 so fp32r
runs at full PE rate.

Host epilogue: out[token] += combine_weight * y_expert[token] (scatter-add).
"""

import math
import numpy as np

import concourse.mybir as mybir
import concourse.tile as tile
from concourse import bacc
from concourse.bass_utils import run_bass_kernel_spmd

E = 8          # experts
TOPK = 2
H = 2048       # hidden
F = 5632       # ffn intermediate
P = 128
N_KT = H // P  # 16 k-subtiles over hidden
N_FT = F // P  # 44 f-tiles
N_HT = H // P  # 16 h-tiles
N_G = 4        # f-tile groups
G_FT = N_FT // N_G  # 11 f-tiles per group

f32 = mybir.dt.float32
f32r = mybir.dt.float32r

# exposed for test.py (profile/exec time inspection)
LAST_RESULTS = None
LAST_NC = None
LAST_IN_MAPS = None


def _route(x, gate_w):
    """numpy float32 router matching the jax reference (softmax/top2/renorm)."""
    logits = x @ gate_w
    m = logits.max(axis=-1, keepdims=True)
    ex = np.exp(logits - m)
    probs = ex / ex.sum(axis=-1, keepdims=True)
    order = np.argsort(-probs, axis=-1, kind="stable")
    top_idx = order[:, :TOPK]
    top_p = np.take_along_axis(probs, top_idx, axis=-1)
    top_w = top_p / top_p.sum(axis=-1, keepdims=True)
    return top_idx, top_w.astype(np.float32)


def _chunks(C):
    ncc = max(1, math.ceil(C / 512))
    out = []
    pos = 0
    for i in range(ncc):
        w = min(C - pos, math.ceil(C / ncc / 2) * 2)
        out.append((pos, w))
        pos += w
    assert pos == C
    return out


def _build(C, use_bf16=False):
    chunk_slices = _chunks(C)
    cmax = max(w for _, w in chunk_slices)
    mdt = mybir.dt.bfloat16 if use_bf16 else f32
    rdt = mybir.dt.bfloat16 if use_bf16 else f32r

    nc = bacc.Bacc("TRN2", target_bir_lowering=False, debug=False,
                   enable_asserts=False, num_devices=E)
    xp = nc.dram_tensor("xp", [P, N_KT, C], mdt, kind="ExternalInput").ap()
    w1p = nc.dram_tensor("w1p", [N_FT, P, N_KT, P], mdt, kind="ExternalInput").ap()
    w3p = nc.dram_tensor("w3p", [N_FT, P, N_KT, P], mdt, kind="ExternalInput").ap()
    w2p = nc.dram_tensor("w2p", [N_G, N_HT, P, G_FT, P], mdt, kind="ExternalInput").ap()
    outp = nc.dram_tensor("outp", [P, N_HT, C], f32, kind="ExternalOutput").ap()

    with tile.TileContext(nc) as tc:
        with tc.tile_pool(name="xt", bufs=1) as xt_pool, \
             tc.tile_pool(name="w13", bufs=2) as w13_pool, \
             tc.tile_pool(name="w2", bufs=3) as w2_pool, \
             tc.tile_pool(name="act", bufs=2) as act_pool, \
             tc.tile_pool(name="tmp", bufs=3) as tmp_pool, \
             tc.tile_pool(name="out", bufs=1) as out_pool, \
             tc.tile_pool(name="psum", bufs=2, space="PSUM") as psum_pool:

            xt = xt_pool.tile([P, N_KT, C], rdt)
            nc.sync.dma_start(out=xt[:], in_=xp[:].bitcast(rdt))
            outt = out_pool.tile([P, N_HT, C], f32)

            for g in range(N_G):
                a_t = act_pool.tile([P, G_FT, C], rdt, tag="act")
                for j in range(G_FT):
                    ft = g * G_FT + j
                    w1b = w13_pool.tile([P, N_KT, P], rdt, tag="w1b")
                    nc.sync.dma_start(out=w1b[:], in_=w1p[ft].bitcast(rdt))
                    w3b = w13_pool.tile([P, N_KT, P], rdt, tag="w3b")
                    nc.sync.dma_start(out=w3b[:], in_=w3p[ft].bitcast(rdt))
                    for c0, cw in chunk_slices:
                        ps1 = psum_pool.tile([P, cmax], f32, tag="ps1", name="ps1")
                        ps1 = ps1[:, :cw]
                        ps3 = psum_pool.tile([P, cmax], f32, tag="ps3", name="ps3")
                        ps3 = ps3[:, :cw]
                        for k in range(N_KT):
                            nc.tensor.matmul(ps1, lhsT=w1b[:, k, :],
                                             rhs=xt[:, k, c0:c0 + cw],
                                             start=(k == 0), stop=(k == N_KT - 1))
                        for k in range(N_KT):
                            nc.tensor.matmul(ps3, lhsT=w3b[:, k, :],
                                             rhs=xt[:, k, c0:c0 + cw],
                                             start=(k == 0), stop=(k == N_KT - 1))
                        st = tmp_pool.tile([P, cmax], f32, tag="silu", name="st")
                        st = st[:, :cw]
                        nc.scalar.activation(st, ps1,
                                             mybir.ActivationFunctionType.Silu)
                        nc.vector.tensor_mul(a_t[:, j, c0:c0 + cw], st, ps3)
                for h in range(N_HT):
                    w2b = w2_pool.tile([P, G_FT, P], rdt, tag="w2b")
                    nc.sync.dma_start(out=w2b[:], in_=w2p[g, h].bitcast(rdt))
                    for c0, cw in chunk_slices:
                        pso = psum_pool.tile([P, cmax], f32, tag="pso", name="pso")
                        pso = pso[:, :cw]
                        for j in range(G_FT):
                            nc.tensor.matmul(pso, lhsT=w2b[:, j, :],
                                             rhs=a_t[:, j, c0:c0 + cw],
                                             start=(j == 0), stop=(j == G_FT - 1))
                        dst = outt[:, h, c0:c0 + cw]
                        if g == 0:
                            nc.vector.tensor_copy(dst, pso)
                        else:
                            nc.vector.tensor_add(dst, dst, pso)

            nc.sync.dma_start(out=outp[:], in_=outt[:])

    nc.compile()
    return nc


def kernel(hidden_states, gate_w, w1, w3, w2):
    global LAST_RESULTS
    import os as _os
    use_bf16 = _os.environ.get("MOE_DTYPE", "f32r") == "bf16"
    B, S, _ = hidden_states.shape
    x = np.ascontiguousarray(hidden_states.reshape(-1, H), dtype=np.float32)
    gate_w = np.asarray(gate_w, dtype=np.float32)
    w1 = np.asarray(w1, dtype=np.float32)
    w3 = np.asarray(w3, dtype=np.float32)
    w2 = np.asarray(w2, dtype=np.float32)
    T = x.shape[0]

    top_idx, top_w = _route(x, gate_w)

    idx_e, cw_e = [], []
    for e in range(E):
        sel = top_idx == e                     # [T, K] bool; <=1 True per row
        tok = np.nonzero(sel.any(axis=1))[0]
        wgt = top_w[tok, np.argmax(sel[tok], axis=1)]
        idx_e.append(tok)
        cw_e.append(wgt.astype(np.float32))

    C = max(8, max(len(t) for t in idx_e))
    C = math.ceil(C / 2) * 2

    if use_bf16:
        import ml_dtypes
        pack_dt = ml_dtypes.bfloat16
    else:
        pack_dt = np.float32

    in_maps = []
    for e in range(E):
        tok = idx_e[e]
        xpk = np.zeros((P, N_KT, C), dtype=pack_dt)
        if len(tok):
            xT = np.ascontiguousarray(x[tok].T)              # [H, C_e]
            xpk[:, :, :len(tok)] = xT.reshape(N_KT, P, len(tok)).transpose(1, 0, 2)
        w1pk = np.ascontiguousarray(
            w1[e].reshape(N_KT, P, N_FT, P).transpose(2, 1, 0, 3).astype(pack_dt))
        w3pk = np.ascontiguousarray(
            w3[e].reshape(N_KT, P, N_FT, P).transpose(2, 1, 0, 3).astype(pack_dt))
        w2pk = np.ascontiguousarray(
            w2[e].reshape(N_G, G_FT, P, N_HT, P).transpose(0, 3, 2, 1, 4).astype(pack_dt))
        in_maps.append({"xp": xpk, "w1p": w1pk, "w3p": w3pk, "w2p": w2pk})

    # the NTFF trace path needs antenv.axon_hooks, absent in this container;
    # force it off so a stray BASS_TRACE env can't break execution
    _os.environ["BASS_NEVER_TRACE"] = "1"
    nc = _build(C, use_bf16=use_bf16)
    res = run_bass_kernel_spmd(nc, in_maps, list(range(E)))
    global LAST_NC, LAST_IN_MAPS
    LAST_RESULTS = res
    LAST_NC = nc
    LAST_IN_MAPS = in_maps

    out = np.zeros((T, H), dtype=np.float32)
    for e in range(E):
        tok = idx_e[e]
        if not len(tok):
            continue
        y = res.results[e]["outp"]                           # [P, N_HT, C]
        y = y.transpose(2, 1, 0).reshape(C, H)[:len(tok)]    # [C_e, H]
        out[tok] += cw_e[e][:, None] * y
    return out.reshape(B, S, H)


# ---------------------------------------------------------------------------
# Timing utility (test-only): re-execute the compiled program on the 8 cores
# with device-resident inputs and no donation, returning per-call wall times.
# ---------------------------------------------------------------------------

def measure_exec(nc, in_maps, iters=6):
    import time as _time
    import jax
    import numpy as _np
    from jax.experimental.shard_map import shard_map
    from jax.sharding import Mesh, PartitionSpec
    from concourse import bass2jax as _b2j
    from concourse.bass2jax import _bass_exec_p, partition_id_tensor

    _b2j.install_neuronx_cc_hook()
    n_cores = len(in_maps)
    partition_name = nc.partition_id_tensor.name if nc.partition_id_tensor else None

    in_names, out_names, out_avals, zero_outs = [], [], [], []
    import concourse.mybir as _mybir
    for alloc in nc.m.functions[0].allocations:
        if not isinstance(alloc, _mybir.MemoryLocationSet):
            continue
        name = alloc.memorylocations[0].name
        if alloc.kind == "ExternalInput":
            if name != partition_name:
                in_names.append(name)
        elif alloc.kind == "ExternalOutput":
            shape = tuple(alloc.tensor_shape)
            dtype = _mybir.dt.np(alloc.dtype)
            out_names.append(name)
            out_avals.append(jax.core.ShapedArray(shape, dtype))
            zero_outs.append(_np.zeros(shape, dtype))
    n_params = len(in_names)
    all_in_names = list(in_names) + list(out_names)
    if partition_name is not None:
        all_in_names.append(partition_name)

    def _body(*args):
        operands = list(args)
        if partition_name is not None:
            operands.append(partition_id_tensor())
        return tuple(_bass_exec_p.bind(
            *operands,
            out_avals=tuple(out_avals),
            in_names=tuple(all_in_names),
            out_names=tuple(out_names),
            lowering_input_output_aliases=(),
            sim_require_finite=True,
            sim_require_nnan=True,
            nc=nc,
        ))

    devices = jax.devices()[:n_cores]
    mesh = Mesh(_np.asarray(devices), ("core",))
    in_specs = (PartitionSpec("core"),) * (n_params + len(out_names))
    out_specs = (PartitionSpec("core"),) * len(out_names)
    fn = jax.jit(shard_map(_body, mesh=mesh, in_specs=in_specs,
                           out_specs=out_specs, check_rep=False),
                 keep_unused=True)

    from jax.sharding import NamedSharding
    sh = NamedSharding(mesh, PartitionSpec("core"))
    concat_in = [
        jax.device_put(_np.concatenate([_np.asarray(in_maps[c][nm])
                                        for c in range(n_cores)], axis=0), sh)
        for nm in in_names
    ]
    concat_zero = [
        jax.device_put(_np.zeros((n_cores * z.shape[0], *z.shape[1:]), z.dtype), sh)
        for z in zero_outs
    ]
    times = []
    for _ in range(iters):
        t0 = _time.time()
        out = fn(*concat_in, *concat_zero)
        jax.block_until_ready(out)
        times.append(_time.time() - t0)
    return times


def measure_exec_repeat(nc, in_maps, reps=(1, 17)):
    """Time R chained executions of the program inside ONE jit dispatch.
    Per-iteration HW time = (t[R2] - t[R1]) / (R2 - R1), dispatch cancels."""
    import time as _time
    import jax
    import numpy as _np
    from jax.experimental.shard_map import shard_map
    from jax.sharding import Mesh, PartitionSpec, NamedSharding
    from concourse import bass2jax as _b2j
    from concourse.bass2jax import _bass_exec_p, partition_id_tensor
    import concourse.mybir as _mybir

    _b2j.install_neuronx_cc_hook()
    n_cores = len(in_maps)
    partition_name = nc.partition_id_tensor.name if nc.partition_id_tensor else None

    in_names, out_names, out_avals, zero_outs = [], [], [], []
    for alloc in nc.m.functions[0].allocations:
        if not isinstance(alloc, _mybir.MemoryLocationSet):
            continue
        name = alloc.memorylocations[0].name
        if alloc.kind == "ExternalInput":
            if name != partition_name:
                in_names.append(name)
        elif alloc.kind == "ExternalOutput":
            shape = tuple(alloc.tensor_shape)
            dtype = _mybir.dt.np(alloc.dtype)
            out_names.append(name)
            out_avals.append(jax.core.ShapedArray(shape, dtype))
            zero_outs.append(_np.zeros(shape, dtype))
    n_params = len(in_names)
    all_in_names = list(in_names) + list(out_names)
    if partition_name is not None:
        all_in_names.append(partition_name)

    def make_body(R):
        n_out = len(out_names)

        def _body(*args):
            ins = list(args[:n_params])
            outs = tuple(args[n_params:n_params + n_out])
            for _ in range(R):
                # feed prev outs as the output-seed operands: breaks CSE and
                # forces serial execution of the R copies
                operands = ins + list(outs)
                if partition_name is not None:
                    operands.append(partition_id_tensor())
                outs = _bass_exec_p.bind(
                    *operands,
                    out_avals=tuple(out_avals),
                    in_names=tuple(all_in_names),
                    out_names=tuple(out_names),
                    lowering_input_output_aliases=(),
                    sim_require_finite=True,
                    sim_require_nnan=True,
                    nc=nc,
                )
                outs = tuple(outs)
            return tuple(outs)
        return _body

    devices = jax.devices()[:n_cores]
    mesh = Mesh(_np.asarray(devices), ("core",))
    in_specs = (PartitionSpec("core"),) * (n_params + len(out_names))
    out_specs = (PartitionSpec("core"),) * len(out_names)
    sh = NamedSharding(mesh, PartitionSpec("core"))
    concat_in = [
        jax.device_put(_np.concatenate([_np.asarray(in_maps[c][nm])
                                        for c in range(n_cores)], axis=0), sh)
        for nm in in_names
    ]
    concat_zero = [
        jax.device_put(_np.zeros((n_cores * z.shape[0], *z.shape[1:]), z.dtype), sh)
        for z in zero_outs
    ]

    results = {}
    for R in reps:
        fn = jax.jit(shard_map(make_body(R), mesh=mesh, in_specs=in_specs,
                               out_specs=out_specs, check_rep=False),
                     keep_unused=True)
        ts = []
        for _ in range(4):
            t0 = _time.time()
            out = fn(*concat_in, *concat_zero)
            jax.block_until_ready(out)
            ts.append(_time.time() - t0)
        results[R] = min(ts[1:])
    (r1, r2) = reps
    per_iter = (results[r2] - results[r1]) / (r2 - r1)
    return results, per_iter


def make_exec_fn(nc, in_maps):
    """Build a jitted 8-core executor over device-resident inputs for timing."""
    import jax
    import numpy as _np
    from jax.experimental.shard_map import shard_map
    from jax.sharding import Mesh, PartitionSpec, NamedSharding
    from concourse import bass2jax as _b2j
    from concourse.bass2jax import _bass_exec_p, partition_id_tensor
    import concourse.mybir as _mybir

    _b2j.install_neuronx_cc_hook()
    n_cores = len(in_maps)
    partition_name = nc.partition_id_tensor.name if nc.partition_id_tensor else None
    in_names, out_names, out_avals, zero_outs = [], [], [], []
    for alloc in nc.m.functions[0].allocations:
        if not isinstance(alloc, _mybir.MemoryLocationSet):
            continue
        name = alloc.memorylocations[0].name
        if alloc.kind == "ExternalInput":
            if name != partition_name:
                in_names.append(name)
        elif alloc.kind == "ExternalOutput":
            shape = tuple(alloc.tensor_shape)
            dtype = _mybir.dt.np(alloc.dtype)
            out_names.append(name)
            out_avals.append(jax.core.ShapedArray(shape, dtype))
            zero_outs.append(_np.zeros(shape, dtype))
    n_params = len(in_names)
    all_in_names = list(in_names) + list(out_names)
    if partition_name is not None:
        all_in_names.append(partition_name)

    def _body(*args):
        operands = list(args)
        if partition_name is not None:
            operands.append(partition_id_tensor())
        return tuple(_bass_exec_p.bind(
            *operands, out_avals=tuple(out_avals), in_names=tuple(all_in_names),
            out_names=tuple(out_names), lowering_input_output_aliases=(),
            sim_require_finite=True, sim_require_nnan=True, nc=nc))

    devices = jax.devices()[:n_cores]
    mesh = Mesh(_np.asarray(devices), ("core",))
    sh = NamedSharding(mesh, PartitionSpec("core"))
    fn = jax.jit(
        shard_map(_body, mesh=mesh,
                  in_specs=(PartitionSpec("core"),) * (n_params + len(out_names)),
                  out_specs=(PartitionSpec("core"),) * len(out_names),
                  check_rep=False),
        keep_unused=True)
    concat_in = [jax.device_put(_np.concatenate(
        [_np.asarray(in_maps[c][nm]) for c in range(n_cores)], axis=0), sh)
        for nm in in_names]
    concat_zero = [jax.device_put(
        _np.zeros((n_cores * z.shape[0], *z.shape[1:]), z.dtype), sh)
        for z in zero_outs]
    return fn, (*concat_in, *concat_zero)


def async_slope(nc, in_maps, n_lo=16, n_hi=96, tries=6):
    """Per-execution time from the slope of N pipelined async dispatches.
    min-of-tries on both ends rejects shared-host contention spikes."""
    import time as _time
    import jax
    fn, args = make_exec_fn(nc, in_maps)

    def run_n(n):
        t0 = _time.time()
        outs = [fn(*args) for _ in range(n)]
        jax.block_until_ready(outs)
        return _time.time() - t0

    run_n(1)  # warm
    t_lo = min(run_n(n_lo) for _ in range(tries))
    t_hi = min(run_n(n_hi) for _ in range(tries))
    return (t_hi - t_lo) / (n_hi - n_lo)

